# revision 9
# baseline (speedup 1.0000x reference)
"""CrossAttention kernel for 8 Trainium2 NeuronCores.

Sharding: batch (4) x query-row-half (2) -> 8 shards, one per core. Each core
computes the full cross-attention for its 1024 query rows of one batch:
Q/K/V projections, 8 heads of attention, and the output projection. K/V
projections are recomputed by both cores sharing a batch (20% extra flops)
in exchange for zero collectives and a pure-SPMD single NEFF.

Layout trick: x and context are transposed on the host so the contraction
dim (feature dim) lands on SBUF partitions with fast contiguous DMAs; all
device matmuls then run without any on-chip transposes:
  QT = Wq.T @ xT      (i on partitions)     KT = Wk.T @ ctxT
  V  = ctxT.T @ Wv    (natural [nk, i])
  ST_h = KT_h @ QT_h  ([nk, nq], K=64, head pairs packed in PE row groups)
  P = exp(ST * scale) (no max-subtraction; logits are ~N(0,1), safe range)
  O^T_h | den_h = [V_h | ones].T @ P  (denominator rides free in the M dim)
  Y = (O^T/den).T @ Wo + bo

All matmuls run as float32r (fp32 bit layout, reduced-precision multiply,
full-rate 1 cyc/row at free-dim >= 256).
"""

import numpy as np

HEADS = 8
DIM_HEAD = 64
SCALE = DIM_HEAD ** -0.5
B, NQ, DQ = 4, 2048, 512
NK, DC = 1024, 768
INNER = HEADS * DIM_HEAD  # 512
NQH = NQ // 2             # query rows per core
N_CORES = 8
P = 128

_PROG_CACHE = {}


def _build_program():
    import concourse.bacc as bacc
    import concourse.tile as tile
    from concourse import mybir
    from concourse.bass import ts, ds

    f32 = mybir.dt.float32
    f32r = mybir.dt.float32r
    Exp = mybir.ActivationFunctionType.Exp

    nc = bacc.Bacc(
        "TRN2",
        target_bir_lowering=False,
        debug=False,
        num_devices=N_CORES,
    )

    xT_d = nc.dram_tensor("xT", [DQ, NQH], f32r, kind="ExternalInput")
    ctxT_d = nc.dram_tensor("ctxT", [DC, NK], f32r, kind="ExternalInput")
    Wq_d = nc.dram_tensor("Wq", [DQ, INNER], f32r, kind="ExternalInput")
    Wk_d = nc.dram_tensor("Wk", [DC, INNER], f32r, kind="ExternalInput")
    Wv_d = nc.dram_tensor("Wv", [DC, INNER], f32r, kind="ExternalInput")
    Wo_d = nc.dram_tensor("Wo", [INNER, DQ], f32r, kind="ExternalInput")
    bo_d = nc.dram_tensor("bo", [DQ], f32, kind="ExternalInput")
    ones_d = nc.dram_tensor("ones", [64], f32r, kind="ExternalInput")
    Y_d = nc.dram_tensor("Y", [NQH, DQ], f32, kind="ExternalOutput")

    KQ = DQ // P   # 4  k-tiles for x-side contraction
    KC = DC // P   # 6  k-tiles for context-side contraction
    KI = INNER // P  # 4 k-tiles for inner-dim contraction
    NQT = NQH // P   # 8  query row tiles
    NKT = NK // P    # 8  key row tiles
    NCH = NQH // 512  # 2 nq chunks of 512

    with tile.TileContext(nc) as tc:
        with (
            tc.tile_pool(name="big", bufs=2) as big,
            tc.tile_pool(name="consts", bufs=1) as consts,
            tc.tile_pool(name="ps", bufs=2, space="PSUM") as ps,
            tc.tile_pool(name="rec", bufs=2) as recp,
            tc.tile_pool(name="yp", bufs=2) as yp,
        ):
            # ---- staged inputs ----
            ctx_sb = big.tile([P, KC, NK], f32r, tag="big")
            nc.sync.dma_start(
                out=ctx_sb, in_=ctxT_d.ap().rearrange("(ko p) n -> p ko n", p=P)
            )
            Wk_sb = consts.tile([P, KC, INNER], f32r, tag="wk")
            nc.sync.dma_start(
                out=Wk_sb, in_=Wk_d.ap().rearrange("(ko p) i -> p ko i", p=P)
            )
            Wq_sb = consts.tile([P, KQ, INNER], f32r, tag="wqo")
            nc.sync.dma_start(
                out=Wq_sb, in_=Wq_d.ap().rearrange("(ko p) i -> p ko i", p=P)
            )
            xT_sb = big.tile([P, KQ, NQH], f32r, tag="big")
            nc.sync.dma_start(
                out=xT_sb, in_=xT_d.ap().rearrange("(ko p) n -> p ko n", p=P)
            )
            Wv_sb = consts.tile([P, KC, INNER], f32r, tag="wv")
            nc.sync.dma_start(
                out=Wv_sb, in_=Wv_d.ap().rearrange("(ko p) i -> p ko i", p=P)
            )
            bo_sb = consts.tile([P, DQ], f32, tag="bo")
            nc.sync.dma_start(
                out=bo_sb, in_=bo_d.ap().unsqueeze(0).to_broadcast((P, DQ))
            )

            KT_sb = consts.tile([P, KI, NQ // 2], f32r, tag="kt")  # [i, nk] 4x1024
            QT_sb = consts.tile([P, KI, NQH], f32r, tag="qt")      # [i, nq]
            # V in natural [nk, i] layout padded per head to 128 cols:
            # even head h: cols h*128+[0:64]=V_h, [64:128]=ones
            # odd  head h: cols h*128+[0:64]=ones, [64:128]=V_h
            V_sb = consts.tile([P, NKT, HEADS * P], f32r, tag="v")
            OT_sb = consts.tile([P, KI, NQH], f32r, tag="ot")      # [i, nq]

            v3 = V_sb.rearrange("p t (j y) -> p (t j) y", j=4)  # [128, 32, 256]
            ones_bc = (
                ones_d.ap().unsqueeze(0).unsqueeze(0)
                .to_broadcast((P, NKT * 4, 64))
            )
            nc.sync.dma_start(out=v3[:, :, 64:128], in_=ones_bc)   # even-head ones
            nc.sync.dma_start(out=v3[:, :, 128:192], in_=ones_bc)  # odd-head ones

            # ---- K projection: KT[i, nk] ----
            for m in range(KI):
                for c in range(NK // 512):
                    psk = ps.tile([P, 512], f32, tag="mm")
                    for k in range(KC):
                        nc.tensor.matmul(
                            psk,
                            lhsT=Wk_sb[:, k, ts(m, P)],
                            rhs=ctx_sb[:, k, ds(c * 512, 512)],
                            start=(k == 0),
                            stop=(k == KC - 1),
                        )
                    nc.vector.tensor_copy(KT_sb[:, m, ds(c * 512, 512)], psk)

            # ---- Q projection: QT[i, nq] ----
            for m in range(KI):
                for c in range(NCH):
                    psq = ps.tile([P, 512], f32, tag="mm")
                    for k in range(KQ):
                        nc.tensor.matmul(
                            psq,
                            lhsT=Wq_sb[:, k, ts(m, P)],
                            rhs=xT_sb[:, k, ds(c * 512, 512)],
                            start=(k == 0),
                            stop=(k == KQ - 1),
                        )
                    nc.vector.tensor_copy(QT_sb[:, m, ds(c * 512, 512)], psq)

            # ---- V projection: V[nk, i] scattered into padded head layout ----
            for t in range(NKT):
                psv = ps.tile([P, 512], f32, tag="mm")
                for k in range(KC):
                    nc.tensor.matmul(
                        psv,
                        lhsT=ctx_sb[:, k, ts(t, P)],
                        rhs=Wv_sb[:, k, :],
                        start=(k == 0),
                        stop=(k == KC - 1),
                    )
                pv4 = psv.rearrange("p (j x) -> p j x", j=4)  # x = 128
                dv4 = V_sb[:, t, :].rearrange("p (j y) -> p j y", j=4)  # y = 256
                nc.vector.tensor_copy(dv4[:, :, 0:64], pv4[:, :, 0:64])
                nc.vector.tensor_copy(dv4[:, :, 192:256], pv4[:, :, 64:128])

            # ---- attention, head pairs packed in PE row groups ----
            T_GROUPS = [(0, 3), (3, 3), (6, 2)]
            for j in range(HEADS // 2):
                for c in range(NCH):
                    eA = big.tile([P, NKT, 512], f32r, tag="big")
                    eB = big.tile([P, NKT, 512], f32r, tag="big")
                    for t0, tn in T_GROUPS:
                        psA = ps.tile([P, 3, 512], f32, tag="s")
                        psB = ps.tile([P, 3, 512], f32, tag="s")
                        for i in range(tn):
                            t = t0 + i
                            nc.tensor.matmul(
                                psA[:, i, :],
                                lhsT=KT_sb[0:64, j, ts(t, P)],
                                rhs=QT_sb[0:64, j, ds(c * 512, 512)],
                                start=True,
                                stop=True,
                            )
                            nc.tensor.matmul(
                                psB[:, i, :],
                                lhsT=KT_sb[64:128, j, ts(t, P)],
                                rhs=QT_sb[64:128, j, ds(c * 512, 512)],
                                start=True,
                                stop=True,
                            )
                        nc.scalar.activation(
                            out=eA[:, t0:t0 + tn, :], in_=psA[:, 0:tn, :],
                            func=Exp, scale=SCALE,
                        )
                        nc.scalar.activation(
                            out=eB[:, t0:t0 + tn, :], in_=psB[:, 0:tn, :],
                            func=Exp, scale=SCALE,
                        )
                    for h, e in ((2 * j, eA), (2 * j + 1, eB)):
                        po = ps.tile([P, 512], f32, tag="mm")
                        for t in range(NKT):
                            nc.tensor.matmul(
                                po,
                                lhsT=V_sb[:, t, ds(h * P, P)],
                                rhs=e[:, t, :],
                                start=(t == 0),
                                stop=(t == NKT - 1),
                            )
                        rc = recp.tile([P, 512], f32, tag="rec")
                        if h % 2 == 0:
                            # O^T rows 0:64, replicated denominator rows 64:128
                            nc.vector.reciprocal(rc[64:128, :], po[64:128, :])
                            nc.sync.dma_start(out=rc[0:64, :], in_=rc[64:128, :])
                            nc.vector.tensor_tensor(
                                OT_sb[0:64, j, ds(c * 512, 512)],
                                po[0:64, :],
                                rc[0:64, :],
                                op=mybir.AluOpType.mult,
                            )
                        else:
                            nc.vector.reciprocal(rc[0:64, :], po[0:64, :])
                            nc.sync.dma_start(out=rc[64:128, :], in_=rc[0:64, :])
                            nc.vector.tensor_tensor(
                                OT_sb[64:128, j, ds(c * 512, 512)],
                                po[64:128, :],
                                rc[64:128, :],
                                op=mybir.AluOpType.mult,
                            )

            # ---- output projection: Y = OT.T @ Wo + bo ----
            Wo_sb = consts.tile([P, KI, DQ], f32r, tag="wqo")
            nc.sync.dma_start(
                out=Wo_sb, in_=Wo_d.ap().rearrange("(ko p) i -> p ko i", p=P)
            )
            for m in range(NQT):
                psy = ps.tile([P, 512], f32, tag="mm")
                for k in range(KI):
                    nc.tensor.matmul(
                        psy,
                        lhsT=OT_sb[:, k, ts(m, P)],
                        rhs=Wo_sb[:, k, :],
                        start=(k == 0),
                        stop=(k == KI - 1),
                    )
                y_t = yp.tile([P, DQ], f32, tag="y")
                nc.vector.tensor_tensor(y_t, psy, bo_sb, op=mybir.AluOpType.add)
                nc.sync.dma_start(out=Y_d.ap()[ts(m, P), :], in_=y_t)

    nc.finalize()
    return nc


def _get_program():
    if "nc" not in _PROG_CACHE:
        _PROG_CACHE["nc"] = _build_program()
    return _PROG_CACHE["nc"]


def kernel(x, context, Wq, Wk, Wv, Wo, bo, **_unused):
    from concourse.bass_utils import run_bass_kernel_spmd

    x = np.asarray(x, dtype=np.float32)
    context = np.asarray(context, dtype=np.float32)
    Wq = np.ascontiguousarray(np.asarray(Wq, dtype=np.float32))
    Wk = np.ascontiguousarray(np.asarray(Wk, dtype=np.float32))
    Wv = np.ascontiguousarray(np.asarray(Wv, dtype=np.float32))
    Wo = np.ascontiguousarray(np.asarray(Wo, dtype=np.float32))
    bo = np.ascontiguousarray(np.asarray(bo, dtype=np.float32))

    nc = _get_program()
    in_maps = []
    for core in range(N_CORES):
        b, half = divmod(core, 2)
        xs = np.ascontiguousarray(x[b, half * NQH:(half + 1) * NQH, :].T)
        cs = np.ascontiguousarray(context[b].T)
        in_maps.append(
            {"xT": xs, "ctxT": cs, "Wq": Wq, "Wk": Wk, "Wv": Wv, "Wo": Wo,
             "bo": bo, "ones": np.ones(64, np.float32)}
        )

    res = run_bass_kernel_spmd(nc, in_maps, core_ids=list(range(N_CORES)))

    out = np.empty((B, NQ, DQ), np.float32)
    for core in range(N_CORES):
        b, half = divmod(core, 2)
        out[b, half * NQH:(half + 1) * NQH, :] = res.results[core]["Y"]
    return out


# revision 11
# speedup vs baseline: 1.0208x; 1.0208x over previous
"""CrossAttention kernel for 8 Trainium2 NeuronCores.

Sharding: batch (4) x query-row-half (2) -> 8 shards, one per core. Each core
computes the full cross-attention for its 1024 query rows of one batch:
Q/K/V projections, 8 heads of attention, and the output projection. K/V
projections are recomputed by both cores sharing a batch (20% extra flops)
in exchange for zero collectives and a pure-SPMD single NEFF.

Layout trick: x and context are transposed on the host so the contraction
dim (feature dim) lands on SBUF partitions with fast contiguous DMAs; all
device matmuls then run without any on-chip transposes:
  QT = Wq.T @ xT      (i on partitions)     KT = Wk.T @ ctxT
  V  = ctxT.T @ Wv    (natural [nk, i])
  ST_h = KT_h @ QT_h  ([nk, nq], K=64, head pairs packed in PE row groups)
  P = exp(ST * scale) (no max-subtraction; logits are ~N(0,1), safe range)
  O^T_h | den_h = [V_h | ones].T @ P  (denominator rides free in the M dim)
  Y = (O^T/den).T @ Wo + bo

All matmuls run as float32r (fp32 bit layout, reduced-precision multiply,
full-rate 1 cyc/row at free-dim >= 256).
"""

import numpy as np

HEADS = 8
DIM_HEAD = 64
SCALE = DIM_HEAD ** -0.5
B, NQ, DQ = 4, 2048, 512
NK, DC = 1024, 768
INNER = HEADS * DIM_HEAD  # 512
NQH = NQ // 2             # query rows per core
N_CORES = 8
P = 128

_PROG_CACHE = {}


def _build_program():
    import concourse.bacc as bacc
    import concourse.tile as tile
    from concourse import mybir
    from concourse.bass import ts, ds

    f32 = mybir.dt.float32
    f32r = mybir.dt.float32r
    Exp = mybir.ActivationFunctionType.Exp

    nc = bacc.Bacc(
        "TRN2",
        target_bir_lowering=False,
        debug=False,
        num_devices=N_CORES,
    )

    xT_d = nc.dram_tensor("xT", [DQ, NQH], f32r, kind="ExternalInput")
    ctxT_d = nc.dram_tensor("ctxT", [DC, NK], f32r, kind="ExternalInput")
    Wq_d = nc.dram_tensor("Wq", [DQ, INNER], f32r, kind="ExternalInput")
    Wk_d = nc.dram_tensor("Wk", [DC, INNER], f32r, kind="ExternalInput")
    Wv_d = nc.dram_tensor("Wv", [DC, INNER], f32r, kind="ExternalInput")
    Wo_d = nc.dram_tensor("Wo", [INNER, DQ], f32r, kind="ExternalInput")
    bo_d = nc.dram_tensor("bo", [DQ], f32, kind="ExternalInput")
    ones_d = nc.dram_tensor("ones", [64], f32r, kind="ExternalInput")
    Y_d = nc.dram_tensor("Y", [NQH, DQ], f32, kind="ExternalOutput")

    KQ = DQ // P   # 4  k-tiles for x-side contraction
    KC = DC // P   # 6  k-tiles for context-side contraction
    KI = INNER // P  # 4 k-tiles for inner-dim contraction
    NQT = NQH // P   # 8  query row tiles
    NKT = NK // P    # 8  key row tiles
    NCH = NQH // 512  # 2 nq chunks of 512

    with tile.TileContext(nc) as tc:
        with (
            tc.tile_pool(name="big", bufs=2) as big,
            tc.tile_pool(name="consts", bufs=1) as consts,
            tc.tile_pool(name="ps", bufs=2, space="PSUM") as ps,
            tc.tile_pool(name="rec", bufs=2) as recp,
            tc.tile_pool(name="yp", bufs=2) as yp,
            tc.tile_pool(name="dram", bufs=2, space="DRAM") as dramp,
        ):
            # ---- staged inputs: spread DMAs over 4 HWDGE queues ----
            engs = [nc.sync, nc.scalar, nc.gpsimd]
            qi = [0]

            def ld(out_ap, in_ap):
                engs[qi[0] % 3].dma_start(out=out_ap, in_=in_ap)
                qi[0] += 1

            Wk_sb = consts.tile([P, KC, INNER], f32r, tag="wk")
            ld(Wk_sb, Wk_d.ap().rearrange("(ko p) i -> p ko i", p=P))
            ctx_sb = big.tile([P, KC, NK], f32r, tag="big")
            ctx_src = ctxT_d.ap().rearrange("(ko p) n -> p ko n", p=P)
            for k in range(KC):
                ld(ctx_sb[:, k:k + 1, :], ctx_src[:, k:k + 1, :])
            Wq_sb = consts.tile([P, KQ, INNER], f32r, tag="wqo")
            ld(Wq_sb, Wq_d.ap().rearrange("(ko p) i -> p ko i", p=P))
            xT_sb = big.tile([P, KQ, NQH], f32r, tag="big")
            xT_src = xT_d.ap().rearrange("(ko p) n -> p ko n", p=P)
            for k in range(KQ):
                ld(xT_sb[:, k:k + 1, :], xT_src[:, k:k + 1, :])
            Wv_sb = consts.tile([P, KC, INNER], f32r, tag="wv")
            ld(Wv_sb, Wv_d.ap().rearrange("(ko p) i -> p ko i", p=P))
            bo_sb = consts.tile([P, DQ], f32, tag="bo")
            ld(bo_sb, bo_d.ap().unsqueeze(0).to_broadcast((P, DQ)))

            KT_sb = consts.tile([P, KI, NQ // 2], f32r, tag="kt")  # [i, nk] 4x1024
            QT_sb = consts.tile([P, KI, NQH], f32r, tag="qt")      # [i, nq]
            # V in natural [nk, i] layout padded per head to 128 cols:
            # even head h: cols h*128+[0:64]=V_h, [64:128]=ones
            # odd  head h: cols h*128+[0:64]=ones, [64:128]=V_h
            V_sb = consts.tile([P, NKT, HEADS * P], f32r, tag="v")
            OT_sb = consts.tile([P, KI, NQH], f32r, tag="ot")      # [i, nq]

            v3 = V_sb.rearrange("p t (j y) -> p (t j) y", j=4)  # [128, 32, 256]
            ones_bc = (
                ones_d.ap().unsqueeze(0).unsqueeze(0)
                .to_broadcast((P, NKT * 4, 64))
            )
            nc.sync.dma_start(out=v3[:, :, 64:128], in_=ones_bc)   # even-head ones
            nc.sync.dma_start(out=v3[:, :, 128:192], in_=ones_bc)  # odd-head ones

            # ---- K projection: KT[i, nk] ----
            for m in range(KI):
                for c in range(NK // 512):
                    psk = ps.tile([P, 512], f32, tag="mm")
                    for k in range(KC):
                        nc.tensor.matmul(
                            psk,
                            lhsT=Wk_sb[:, k, ts(m, P)],
                            rhs=ctx_sb[:, k, ds(c * 512, 512)],
                            start=(k == 0),
                            stop=(k == KC - 1),
                        )
                    nc.vector.tensor_copy(KT_sb[:, m, ds(c * 512, 512)], psk)

            # ---- Q projection: QT[i, nq] ----
            for m in range(KI):
                for c in range(NCH):
                    psq = ps.tile([P, 512], f32, tag="mm")
                    for k in range(KQ):
                        nc.tensor.matmul(
                            psq,
                            lhsT=Wq_sb[:, k, ts(m, P)],
                            rhs=xT_sb[:, k, ds(c * 512, 512)],
                            start=(k == 0),
                            stop=(k == KQ - 1),
                        )
                    nc.vector.tensor_copy(QT_sb[:, m, ds(c * 512, 512)], psq)

            # ---- V projection: V[nk, i] scattered into padded head layout ----
            for t in range(NKT):
                psv = ps.tile([P, 512], f32, tag="mm")
                for k in range(KC):
                    nc.tensor.matmul(
                        psv,
                        lhsT=ctx_sb[:, k, ts(t, P)],
                        rhs=Wv_sb[:, k, :],
                        start=(k == 0),
                        stop=(k == KC - 1),
                    )
                pv4 = psv.rearrange("p (j x) -> p j x", j=4)  # x = 128
                dv4 = V_sb[:, t, :].rearrange("p (j y) -> p j y", j=4)  # y = 256
                nc.vector.tensor_copy(dv4[:, :, 0:64], pv4[:, :, 0:64])
                nc.vector.tensor_copy(dv4[:, :, 192:256], pv4[:, :, 64:128])

            # ---- attention, head pairs packed in PE row groups ----
            T_GROUPS = [(0, 3), (3, 3), (6, 2)]
            for j in range(HEADS // 2):
                for c in range(NCH):
                    eA = big.tile([P, NKT, 512], f32r, tag="big")
                    eB = big.tile([P, NKT, 512], f32r, tag="big")
                    for t0, tn in T_GROUPS:
                        psA = ps.tile([P, 3, 512], f32, tag="s")
                        psB = ps.tile([P, 3, 512], f32, tag="s")
                        for i in range(tn):
                            t = t0 + i
                            nc.tensor.matmul(
                                psA[:, i, :],
                                lhsT=KT_sb[0:64, j, ts(t, P)],
                                rhs=QT_sb[0:64, j, ds(c * 512, 512)],
                                start=True,
                                stop=True,
                            )
                            nc.tensor.matmul(
                                psB[:, i, :],
                                lhsT=KT_sb[64:128, j, ts(t, P)],
                                rhs=QT_sb[64:128, j, ds(c * 512, 512)],
                                start=True,
                                stop=True,
                            )
                        nc.scalar.activation(
                            out=eA[:, t0:t0 + tn, :], in_=psA[:, 0:tn, :],
                            func=Exp, scale=SCALE,
                        )
                        nc.scalar.activation(
                            out=eB[:, t0:t0 + tn, :], in_=psB[:, 0:tn, :],
                            func=Exp, scale=SCALE,
                        )
                    for h, e in ((2 * j, eA), (2 * j + 1, eB)):
                        po = ps.tile([P, 512], f32, tag="mm")
                        for t in range(NKT):
                            nc.tensor.matmul(
                                po,
                                lhsT=V_sb[:, t, ds(h * P, P)],
                                rhs=e[:, t, :],
                                start=(t == 0),
                                stop=(t == NKT - 1),
                            )
                        # evict PSUM immediately; normalize off SBUF
                        o_raw = recp.tile([P, 512], f32, tag="oraw")
                        nc.vector.tensor_copy(o_raw, po)
                        olo, ohi = (0, 64) if h % 2 == 0 else (64, 128)
                        dlo = 64 if h % 2 == 0 else 0
                        # chop den row -> [64, 8] for a cheap reciprocal
                        dg = recp.tile([64, 8], f32, tag="dg")
                        nc.gpsimd.dma_start(
                            out=dg, in_=o_raw[dlo:dlo + 1, :]
                        )
                        rg = recp.tile([64, 8], f32, tag="rg")
                        nc.vector.reciprocal(rg, dg)
                        # broadcast 1/den across partitions via DRAM bounce
                        dsc = dramp.tile([512], f32, tag="ds")
                        nc.gpsimd.dma_start(out=dsc, in_=rg)
                        rb = recp.tile([P, 512], f32, tag="rb")
                        nc.gpsimd.dma_start(
                            out=rb[olo:ohi, :],
                            in_=dsc.unsqueeze(0).to_broadcast((64, 512)),
                        )
                        nc.vector.tensor_tensor(
                            OT_sb[olo:ohi, j, ds(c * 512, 512)],
                            o_raw[olo:ohi, :],
                            rb[olo:ohi, :],
                            op=mybir.AluOpType.mult,
                        )

            # ---- output projection: Y = OT.T @ Wo + bo ----
            Wo_sb = consts.tile([P, KI, DQ], f32r, tag="wqo")
            nc.sync.dma_start(
                out=Wo_sb, in_=Wo_d.ap().rearrange("(ko p) i -> p ko i", p=P)
            )
            for m in range(NQT):
                psy = ps.tile([P, 512], f32, tag="mm")
                for k in range(KI):
                    nc.tensor.matmul(
                        psy,
                        lhsT=OT_sb[:, k, ts(m, P)],
                        rhs=Wo_sb[:, k, :],
                        start=(k == 0),
                        stop=(k == KI - 1),
                    )
                y_t = yp.tile([P, DQ], f32, tag="y")
                nc.vector.tensor_tensor(y_t, psy, bo_sb, op=mybir.AluOpType.add)
                nc.sync.dma_start(out=Y_d.ap()[ts(m, P), :], in_=y_t)

    nc.finalize()
    return nc


def _get_program():
    if "nc" not in _PROG_CACHE:
        _PROG_CACHE["nc"] = _build_program()
    return _PROG_CACHE["nc"]


def kernel(x, context, Wq, Wk, Wv, Wo, bo, **_unused):
    from concourse.bass_utils import run_bass_kernel_spmd

    x = np.asarray(x, dtype=np.float32)
    context = np.asarray(context, dtype=np.float32)
    Wq = np.ascontiguousarray(np.asarray(Wq, dtype=np.float32))
    Wk = np.ascontiguousarray(np.asarray(Wk, dtype=np.float32))
    Wv = np.ascontiguousarray(np.asarray(Wv, dtype=np.float32))
    Wo = np.ascontiguousarray(np.asarray(Wo, dtype=np.float32))
    bo = np.ascontiguousarray(np.asarray(bo, dtype=np.float32))

    nc = _get_program()
    in_maps = []
    for core in range(N_CORES):
        b, half = divmod(core, 2)
        xs = np.ascontiguousarray(x[b, half * NQH:(half + 1) * NQH, :].T)
        cs = np.ascontiguousarray(context[b].T)
        in_maps.append(
            {"xT": xs, "ctxT": cs, "Wq": Wq, "Wk": Wk, "Wv": Wv, "Wo": Wo,
             "bo": bo, "ones": np.ones(64, np.float32)}
        )

    res = run_bass_kernel_spmd(nc, in_maps, core_ids=list(range(N_CORES)))

    out = np.empty((B, NQ, DQ), np.float32)
    for core in range(N_CORES):
        b, half = divmod(core, 2)
        out[b, half * NQH:(half + 1) * NQH, :] = res.results[core]["Y"]
    return out


# revision 13
# speedup vs baseline: 1.4417x; 1.4123x over previous
"""CrossAttention kernel for 8 Trainium2 NeuronCores.

Sharding: batch (4) x query-row-half (2) -> 8 shards, one per core. Each core
computes the full cross-attention for its 1024 query rows of one batch:
Q/K/V projections, 8 heads of attention, and the output projection. K/V
projections are recomputed by both cores sharing a batch (20% extra flops)
in exchange for zero collectives and a pure-SPMD single NEFF.

Layout trick: x and context are transposed on the host so the contraction
dim (feature dim) lands on SBUF partitions with fast contiguous DMAs; all
device matmuls then run without any on-chip transposes:
  QT = Wq.T @ xT      (i on partitions)     KT = Wk.T @ ctxT
  V  = ctxT.T @ Wv    (natural [nk, i])
  ST_h = KT_h @ QT_h  ([nk, nq], K=64, head pairs packed in PE row groups)
  P = exp(ST * scale) (no max-subtraction; logits are ~N(0,1), safe range)
  O^T_h | den_h = [V_h | ones].T @ P  (denominator rides free in the M dim)
  Y = (O^T/den).T @ Wo + bo

All matmuls run as float32r (fp32 bit layout, reduced-precision multiply,
full-rate 1 cyc/row at free-dim >= 256).
"""

import numpy as np

HEADS = 8
DIM_HEAD = 64
SCALE = DIM_HEAD ** -0.5
B, NQ, DQ = 4, 2048, 512
NK, DC = 1024, 768
INNER = HEADS * DIM_HEAD  # 512
NQH = NQ // 2             # query rows per core
N_CORES = 8
P = 128

_PROG_CACHE = {}


def _build_program():
    import concourse.bacc as bacc
    import concourse.bass as bass
    import concourse.tile as tile
    from concourse import mybir
    from concourse.bass import ts, ds

    f32 = mybir.dt.float32
    f32r = mybir.dt.float32r
    Exp = mybir.ActivationFunctionType.Exp

    nc = bacc.Bacc(
        "TRN2",
        target_bir_lowering=False,
        debug=False,
        num_devices=N_CORES,
    )

    xT_d = nc.dram_tensor("xT", [DQ, NQH], f32r, kind="ExternalInput")
    ctxT_d = nc.dram_tensor("ctxT", [DC, NK], f32r, kind="ExternalInput")
    Wq_d = nc.dram_tensor("Wq", [DQ, INNER], f32r, kind="ExternalInput")
    Wk_d = nc.dram_tensor("Wk", [DC, INNER], f32r, kind="ExternalInput")
    Wv_d = nc.dram_tensor("Wv", [DC, INNER], f32r, kind="ExternalInput")
    Wo_d = nc.dram_tensor("Wo", [INNER, DQ], f32r, kind="ExternalInput")
    bo_d = nc.dram_tensor("bo", [DQ], f32, kind="ExternalInput")
    ones_d = nc.dram_tensor("ones", [4, 128], f32r, kind="ExternalInput")
    Y_d = nc.dram_tensor("Y", [NQH, DQ], f32, kind="ExternalOutput")

    KQ = DQ // P   # 4  k-tiles for x-side contraction
    KC = DC // P   # 6  k-tiles for context-side contraction
    KI = INNER // P  # 4 k-tiles for inner-dim contraction
    NQT = NQH // P   # 8  query row tiles
    NKT = NK // P    # 8  key row tiles
    NCH = NQH // 512  # 2 nq chunks of 512

    with tile.TileContext(nc) as tc:
        with (
            tc.tile_pool(name="big", bufs=2) as big,
            tc.tile_pool(name="consts", bufs=1) as consts,
            tc.tile_pool(name="ps", bufs=2, space="PSUM") as ps,
            tc.tile_pool(name="rec", bufs=2) as recp,
            tc.tile_pool(name="yp", bufs=2) as yp,
            tc.tile_pool(name="dram", bufs=2, space="DRAM") as dramp,
        ):
            # ---- staged inputs: spread DMAs over 4 HWDGE queues ----
            engs = [nc.sync, nc.scalar, nc.gpsimd]
            qi = [0]

            def ld(out_ap, in_ap):
                engs[qi[0] % 3].dma_start(out=out_ap, in_=in_ap)
                qi[0] += 1

            Wk_sb = consts.tile([P, KC, INNER], f32r, tag="wk")
            ld(Wk_sb, Wk_d.ap().rearrange("(ko p) i -> p ko i", p=P))
            ctx_sb = big.tile([P, KC, NK], f32r, tag="big")
            ctx_src = ctxT_d.ap().rearrange("(ko p) n -> p ko n", p=P)
            for k in range(KC):
                ld(ctx_sb[:, k:k + 1, :], ctx_src[:, k:k + 1, :])
            Wq_sb = consts.tile([P, KQ, INNER], f32r, tag="wqo")
            ld(Wq_sb, Wq_d.ap().rearrange("(ko p) i -> p ko i", p=P))
            xT_sb = big.tile([P, KQ, NQH], f32r, tag="big")
            xT_src = xT_d.ap().rearrange("(ko p) n -> p ko n", p=P)
            for k in range(KQ):
                ld(xT_sb[:, k:k + 1, :], xT_src[:, k:k + 1, :])
            Wv_sb = consts.tile([P, KC, INNER], f32r, tag="wv")
            ld(Wv_sb, Wv_d.ap().rearrange("(ko p) i -> p ko i", p=P))
            bo_sb = consts.tile([P, DQ], f32, tag="bo")
            ld(bo_sb, bo_d.ap().unsqueeze(0).to_broadcast((P, DQ)))

            KT_sb = consts.tile([P, KI, NQ // 2], f32r, tag="kt")  # [i, nk] 4x1024
            QT_sb = consts.tile([P, KI, NQH], f32r, tag="qt")      # [i, nq]
            # V in natural [nk, i] layout padded per head to 128 cols:
            # even head h: cols h*128+[0:64]=V_h, [64:128]=ones
            # odd  head h: cols h*128+[0:64]=ones, [64:128]=V_h
            # (ones regions merge to cols 64:192 of every 256-col pair block)
            V_sb = consts.tile([P, NKT, HEADS * P], f32r, tag="v")
            OT_sb = consts.tile([P, KI, NQH], f32r, tag="ot")      # [i, nq]

            ones_src = ones_d.ap().unsqueeze(0).to_broadcast((P, 4, 128))
            for t in range(NKT):
                dv4 = V_sb[:, t, :].rearrange("p (j y) -> p j y", j=4)
                engs[t % 3].dma_start(out=dv4[:, :, 64:192], in_=ones_src)

            # ---- K projection: KT[i, nk] ----
            for m in range(KI):
                for c in range(NK // 512):
                    psk = ps.tile([P, 512], f32, tag="mm")
                    for k in range(KC):
                        nc.tensor.matmul(
                            psk,
                            lhsT=Wk_sb[:, k, ts(m, P)],
                            rhs=ctx_sb[:, k, ds(c * 512, 512)],
                            start=(k == 0),
                            stop=(k == KC - 1),
                        )
                    nc.vector.tensor_copy(KT_sb[:, m, ds(c * 512, 512)], psk)

            # ---- Q projection: QT[i, nq] ----
            for m in range(KI):
                for c in range(NCH):
                    psq = ps.tile([P, 512], f32, tag="mm")
                    for k in range(KQ):
                        nc.tensor.matmul(
                            psq,
                            lhsT=Wq_sb[:, k, ts(m, P)],
                            rhs=xT_sb[:, k, ds(c * 512, 512)],
                            start=(k == 0),
                            stop=(k == KQ - 1),
                        )
                    nc.vector.tensor_copy(QT_sb[:, m, ds(c * 512, 512)], psq)

            # ---- V projection: V[nk, i] scattered into padded head layout ----
            for t in range(NKT):
                psv = ps.tile([P, 512], f32, tag="mm")
                for k in range(KC):
                    nc.tensor.matmul(
                        psv,
                        lhsT=ctx_sb[:, k, ts(t, P)],
                        rhs=Wv_sb[:, k, :],
                        start=(k == 0),
                        stop=(k == KC - 1),
                    )
                pv4 = psv.rearrange("p (j x) -> p j x", j=4)  # x = 128
                dv4 = V_sb[:, t, :].rearrange("p (j y) -> p j y", j=4)  # y = 256
                nc.vector.tensor_copy(dv4[:, :, 0:64], pv4[:, :, 0:64])
                nc.vector.tensor_copy(dv4[:, :, 192:256], pv4[:, :, 64:128])

            # ---- attention, head pairs packed in PE row groups ----
            T_GROUPS = [(0, 3), (3, 3), (6, 2)]
            for j in range(HEADS // 2):
                for c in range(NCH):
                    eA = big.tile([P, NKT, 512], f32r, tag="big")
                    eB = big.tile([P, NKT, 512], f32r, tag="big")
                    for t0, tn in T_GROUPS:
                        psA = ps.tile([P, 3, 512], f32, tag="s")
                        psB = ps.tile([P, 3, 512], f32, tag="s")
                        for i in range(tn):
                            t = t0 + i
                            nc.tensor.matmul(
                                psA[:, i, :],
                                lhsT=KT_sb[0:64, j, ts(t, P)],
                                rhs=QT_sb[0:64, j, ds(c * 512, 512)],
                                start=True,
                                stop=True,
                            )
                            nc.tensor.matmul(
                                psB[:, i, :],
                                lhsT=KT_sb[64:128, j, ts(t, P)],
                                rhs=QT_sb[64:128, j, ds(c * 512, 512)],
                                start=True,
                                stop=True,
                            )
                        nc.scalar.activation(
                            out=eA[:, t0:t0 + tn, :], in_=psA[:, 0:tn, :],
                            func=Exp, scale=SCALE,
                        )
                        nc.scalar.activation(
                            out=eB[:, t0:t0 + tn, :], in_=psB[:, 0:tn, :],
                            func=Exp, scale=SCALE,
                        )
                    for h, e in ((2 * j, eA), (2 * j + 1, eB)):
                        po = ps.tile([P, 512], f32, tag="mm")
                        for t in range(NKT):
                            nc.tensor.matmul(
                                po,
                                lhsT=V_sb[:, t, ds(h * P, P)],
                                rhs=e[:, t, :],
                                start=(t == 0),
                                stop=(t == NKT - 1),
                            )
                        # evict PSUM immediately; normalize off SBUF
                        o_raw = recp.tile([P, 512], f32, tag="oraw")
                        nc.vector.tensor_copy(o_raw, po)
                        olo, ohi = (0, 64) if h % 2 == 0 else (64, 128)
                        dlo = 64 if h % 2 == 0 else 0
                        # chop den row -> [64, 8] for a cheap reciprocal
                        dg = recp.tile([64, 8], f32, tag="dg")
                        nc.gpsimd.dma_start(
                            out=dg, in_=o_raw[dlo:dlo + 1, :]
                        )
                        rg = recp.tile([64, 8], f32, tag="rg")
                        nc.vector.reciprocal(rg, dg)
                        # broadcast 1/den across partitions via DRAM bounce
                        dsc = dramp.tile([512], f32, tag="ds")
                        nc.gpsimd.dma_start(out=dsc, in_=rg)
                        rb = recp.tile([P, 512], f32, tag="rb")
                        nc.gpsimd.dma_start(
                            out=rb[olo:ohi, :],
                            in_=dsc.unsqueeze(0).to_broadcast((64, 512)),
                        )
                        nc.vector.tensor_tensor(
                            OT_sb[olo:ohi, j, ds(c * 512, 512)],
                            o_raw[olo:ohi, :],
                            rb[olo:ohi, :],
                            op=mybir.AluOpType.mult,
                        )

            # ---- output projection: Y = OT.T @ Wo + bo ----
            Wo_sb = consts.tile([P, KI, DQ], f32r, tag="wqo")
            nc.sync.dma_start(
                out=Wo_sb, in_=Wo_d.ap().rearrange("(ko p) i -> p ko i", p=P)
            )
            for m in range(NQT):
                psy = ps.tile([P, 512], f32, tag="mm")
                for k in range(KI):
                    nc.tensor.matmul(
                        psy,
                        lhsT=OT_sb[:, k, ts(m, P)],
                        rhs=Wo_sb[:, k, :],
                        start=(k == 0),
                        stop=(k == KI - 1),
                    )
                y_t = yp.tile([P, DQ], f32, tag="y")
                nc.vector.tensor_tensor(y_t, psy, bo_sb, op=mybir.AluOpType.add)
                nc.sync.dma_start(out=Y_d.ap()[ts(m, P), :], in_=y_t)

    nc.finalize()
    return nc


def _get_program():
    if "nc" not in _PROG_CACHE:
        _PROG_CACHE["nc"] = _build_program()
    return _PROG_CACHE["nc"]


def kernel(x, context, Wq, Wk, Wv, Wo, bo, **_unused):
    from concourse.bass_utils import run_bass_kernel_spmd

    x = np.asarray(x, dtype=np.float32)
    context = np.asarray(context, dtype=np.float32)
    Wq = np.ascontiguousarray(np.asarray(Wq, dtype=np.float32))
    Wk = np.ascontiguousarray(np.asarray(Wk, dtype=np.float32))
    Wv = np.ascontiguousarray(np.asarray(Wv, dtype=np.float32))
    Wo = np.ascontiguousarray(np.asarray(Wo, dtype=np.float32))
    bo = np.ascontiguousarray(np.asarray(bo, dtype=np.float32))

    nc = _get_program()
    in_maps = []
    for core in range(N_CORES):
        b, half = divmod(core, 2)
        xs = np.ascontiguousarray(x[b, half * NQH:(half + 1) * NQH, :].T)
        cs = np.ascontiguousarray(context[b].T)
        in_maps.append(
            {"xT": xs, "ctxT": cs, "Wq": Wq, "Wk": Wk, "Wv": Wv, "Wo": Wo,
             "bo": bo, "ones": np.ones((4, 128), np.float32)}
        )

    res = run_bass_kernel_spmd(nc, in_maps, core_ids=list(range(N_CORES)))

    out = np.empty((B, NQ, DQ), np.float32)
    for core in range(N_CORES):
        b, half = divmod(core, 2)
        out[b, half * NQH:(half + 1) * NQH, :] = res.results[core]["Y"]
    return out
